# revision 1
# baseline (speedup 1.0000x reference)
"""AttentionPool Trainium2 kernel.

Computes, for x (B,T,m), W1 (m,m), W2 (m,m), vm (1,m):
    h      = tanh(x @ W1 + vm @ W2)          (B,T,m)
    scores = h @ vm[0]                       (B,T,1)
    w      = softmax(scores, axis=T)
    out    = sum(x * w, axis=T, keepdims)    (B,1,m)

Sharding: data-parallel over B across 8 NeuronCores (2 examples per core);
W1/W2/vm replicated.  Softmax needs no max-subtraction: |scores| <= ||vm||_1
(~13 at this problem scale), safely inside fp32 exp range, so the kernel is
a single streaming pass over x with exp and Z accumulated online.

Per-core dataflow (chunk = 512 rows of T, laid out t = c*512 + p*4 + r so
each DMA descriptor is 4 KiB contiguous):
  DMA x chunk (natural f32)
  -> cast fp16 (GPSIMD)
  -> PE transpose (fp16) -> xT in SBUF (DVE psum->sbuf copy)
  -> h^T = W1.T @ x^T per n-half (PE fp16, W1 stationary)
  -> tanh + per-partition bias (ACT, h^T layout)
  -> scores: s = h^T.T @ vm per 128-t block (PE, h stationary; lands
     t-partitioned in psum)
  -> e = exp(s) (ACT) into per-example e_all
  -> pooling: acc[p,m] += x[p,r,m] * e[p] (DVE scalar_tensor_tensor, f32)
  -> tail per example: Z = sum(e_all) (DVE reduce + PE partition-reduce),
     acc partition-reduce on PE, scale by 1/Z, DMA out.
"""

import numpy as np

import concourse.bass as bass
import concourse.tile as tile
from concourse import bacc, mybir
from concourse.bass_utils import run_bass_kernel_spmd
from concourse.masks import make_identity

FP32 = mybir.dt.float32
FP16 = mybir.dt.float16
AF = mybir.ActivationFunctionType
ALU = mybir.AluOpType

N_CORES = 8
B = 16
B_PER_CORE = B // N_CORES  # 2
T = 8192
M = 256
P = 128
CHUNK = 512          # t rows per chunk
NT = CHUNK // P      # 4 t-tiles (r values) per chunk
NCHUNK = T // CHUNK  # 16 chunks per example


def _build_program() -> bass.Bass:
    nc = bacc.Bacc("TRN2", target_bir_lowering=False, debug=False)

    x = nc.dram_tensor("x", [B_PER_CORE, T, M], FP32, kind="ExternalInput")
    W1 = nc.dram_tensor("W1", [M, M], FP32, kind="ExternalInput")
    W2 = nc.dram_tensor("W2", [M, M], FP32, kind="ExternalInput")
    vm = nc.dram_tensor("vm", [1, M], FP32, kind="ExternalInput")
    out = nc.dram_tensor("out", [B_PER_CORE, M], FP32, kind="ExternalOutput")

    with tile.TileContext(nc) as tc:
        with (
            tc.tile_pool(name="setup", bufs=1) as setup,
            tc.tile_pool(name="xin", bufs=6) as xin_pool,
            tc.tile_pool(name="xbf", bufs=2) as xbf_pool,
            tc.tile_pool(name="xtp", bufs=2, space="PSUM") as xtp_pool,
            tc.tile_pool(name="xts", bufs=2) as xts_pool,
            tc.tile_pool(name="hps", bufs=2, space="PSUM") as hps_pool,
            tc.tile_pool(name="hsb", bufs=2) as hsb_pool,
            tc.tile_pool(name="sps", bufs=1, space="PSUM") as sps_pool,
            tc.tile_pool(name="fps", bufs=1, space="PSUM") as fps_pool,
            tc.tile_pool(name="eee", bufs=2) as e_pool,
            tc.tile_pool(name="acc", bufs=2) as acc_pool,
            tc.tile_pool(name="fin", bufs=2) as fin_pool,
        ):
            # ---------------- setup ----------------
            ident = setup.tile([P, P], FP16)
            make_identity(nc, ident)

            # W1 blocks: w1b[p, mh, n] = W1[mh*128+p, n], cast to fp16
            w1f = setup.tile([P, 2, M], FP32)
            nc.sync.dma_start(out=w1f, in_=W1.rearrange("(a p) n -> p a n", p=P))
            w1b = setup.tile([P, 2, M], FP16)
            nc.vector.tensor_copy(w1b, w1f)

            # W2 blocks (f32, setup only)
            w2f = setup.tile([P, 2, M], FP32)
            nc.sync.dma_start(out=w2f, in_=W2.rearrange("(a p) n -> p a n", p=P))

            # vm transposed: vmt[p, mh] = vm[0, mh*128+p]
            vmt_f = setup.tile([P, 2], FP32)
            nc.sync.dma_start(out=vmt_f, in_=vm[0].rearrange("(a p) -> p a", p=P))
            vmt_b = setup.tile([P, 2], FP16)
            nc.vector.tensor_copy(vmt_b, vmt_f)

            # c = vm @ W2, computed directly transposed: c_sb[p, nh] = c[nh*128+p]
            c_ps = sps_pool.tile([P, 2], FP32, tag="sps")
            for nh in range(2):
                for mh in range(2):
                    nc.tensor.matmul(
                        c_ps[:, nh : nh + 1],
                        lhsT=w2f[:, mh, nh * P : (nh + 1) * P],
                        rhs=vmt_f[:, mh : mh + 1],
                        start=(mh == 0),
                        stop=(mh == 1),
                    )
            c_sb = setup.tile([P, 2], FP32)
            nc.vector.tensor_copy(c_sb, c_ps)

            ones_col = setup.tile([P, 1], FP32)
            nc.vector.memset(ones_col, 1.0)
            ones_row = setup.tile([1, P], FP32)
            nc.vector.memset(ones_row, 1.0)

            # ---------------- main loop ----------------
            for b in range(B_PER_CORE):
                e_all = e_pool.tile([P, NCHUNK * NT], FP32)
                acc = acc_pool.tile([P, M], FP32)
                nc.vector.memset(acc, 0.0)

                for c in range(NCHUNK):
                    # x chunk: xin[p, r, m] = x[b, c*512 + p*4 + r, m]
                    # -> per-partition 4 KiB contiguous DMA descriptors
                    xin = xin_pool.tile([P, NT, M], FP32)
                    nc.sync.dma_start(
                        out=xin,
                        in_=x[b, c * CHUNK : (c + 1) * CHUNK, :].rearrange(
                            "(p r) m -> p r m", p=P
                        ),
                    )

                    # cast to fp16 for the score path
                    xbf = xbf_pool.tile([P, NT, M], FP16)
                    nc.gpsimd.tensor_copy(xbf, xin)

                    # PE transpose -> xtp[q, mh, r, p] = x[t=p*4+r, mh*128+q]
                    xtp = xtp_pool.tile([P, 2, NT, P], FP16)
                    for r in range(NT):
                        for mh in range(2):
                            nc.tensor.transpose(
                                xtp[:, mh, r, :],
                                xbf[:, r, mh * P : (mh + 1) * P],
                                ident,
                            )
                    xts = xts_pool.tile([P, 2, NT, P], FP16)
                    nc.vector.tensor_copy(xts, xtp)

                    # h^T = W1.T @ x^T (per n-half), accumulate over m-halves
                    hps = hps_pool.tile([P, 2, CHUNK], FP32)
                    for nh in range(2):
                        for mh in range(2):
                            nc.tensor.matmul(
                                hps[:, nh, :],
                                lhsT=w1b[:, mh, nh * P : (nh + 1) * P],
                                rhs=xts[:, mh],
                                start=(mh == 0),
                                stop=(mh == 1),
                            )

                    # tanh with per-partition bias c
                    hsb = hsb_pool.tile([P, 2, CHUNK], FP16)
                    for nh in range(2):
                        nc.scalar.activation(
                            hsb[:, nh],
                            hps[:, nh],
                            AF.Tanh,
                            bias=c_sb[:, nh : nh + 1],
                        )

                    # scores: s[q, r] for t = q*4 + r (t-partitioned)
                    sps = sps_pool.tile([P, NT], FP32, tag="sps")
                    for r in range(NT):
                        for nh in range(2):
                            nc.tensor.matmul(
                                sps[:, r : r + 1],
                                lhsT=hsb[:, nh, r * P : (r + 1) * P],
                                rhs=vmt_b[:, nh : nh + 1],
                                start=(nh == 0),
                                stop=(nh == 1),
                            )

                    # e = exp(s) into the per-example e table
                    nc.scalar.activation(
                        e_all[:, c * NT : (c + 1) * NT],
                        sps,
                        AF.Exp,
                    )

                    # pooling: acc[p, m] += x[p, r, m] * e[p, c*4+r]
                    for r in range(NT):
                        nc.vector.scalar_tensor_tensor(
                            out=acc,
                            in0=xin[:, r],
                            scalar=e_all[:, c * NT + r : c * NT + r + 1],
                            in1=acc,
                            op0=ALU.mult,
                            op1=ALU.add,
                        )

                # ---- finalize example ----
                # Z = sum(e_all): free-dim reduce on DVE, partition reduce on PE
                z_red = fin_pool.tile([P, 1], FP32)
                nc.vector.reduce_sum(z_red, e_all, axis=mybir.AxisListType.X)
                z_ps = fps_pool.tile([1, 1], FP32, tag="fps")
                nc.tensor.matmul(z_ps, lhsT=z_red, rhs=ones_col, start=True, stop=True)
                z_sb = fin_pool.tile([1, 1], FP32)
                nc.vector.tensor_copy(z_sb, z_ps)
                # broadcast Z to all partitions, then reciprocal
                zb_ps = fps_pool.tile([P, 1], FP32, tag="fps")
                nc.tensor.matmul(zb_ps, lhsT=ones_row, rhs=z_sb, start=True, stop=True)
                rz = fin_pool.tile([P, 1], FP32)
                nc.vector.reciprocal(rz, zb_ps)
                # partition-reduce acc: outT[q, mh] = sum_p acc[p, mh*128+q]
                outT_ps = fps_pool.tile([P, 2], FP32, tag="fps")
                for mh in range(2):
                    nc.tensor.matmul(
                        outT_ps[:, mh : mh + 1],
                        lhsT=acc[:, mh * P : (mh + 1) * P],
                        rhs=ones_col,
                        start=True,
                        stop=True,
                    )
                outsb = fin_pool.tile([P, 2], FP32)
                nc.vector.tensor_scalar_mul(outsb, outT_ps, rz)
                nc.sync.dma_start(
                    out=out[b].rearrange("(a p) -> p a", p=P), in_=outsb
                )

    return nc


_PROGRAM_CACHE: list = []


def _get_program() -> bass.Bass:
    if not _PROGRAM_CACHE:
        nc = _build_program()
        nc.finalize()
        _PROGRAM_CACHE.append(nc)
    return _PROGRAM_CACHE[0]


def kernel(x, W1, W2, vm):
    x = np.ascontiguousarray(x, dtype=np.float32)
    W1 = np.ascontiguousarray(W1, dtype=np.float32)
    W2 = np.ascontiguousarray(W2, dtype=np.float32)
    vm = np.ascontiguousarray(vm, dtype=np.float32)

    nc = _get_program()
    core_ids = list(range(N_CORES))
    in_maps = [
        {
            "x": x[i * B_PER_CORE : (i + 1) * B_PER_CORE],
            "W1": W1,
            "W2": W2,
            "vm": vm,
        }
        for i in range(N_CORES)
    ]
    res = run_bass_kernel_spmd(nc, in_maps, core_ids)
    out = np.concatenate([res.results[i]["out"] for i in range(N_CORES)], axis=0)
    return out.reshape(B, 1, M)



# revision 6
# speedup vs baseline: 1.9478x; 1.9478x over previous
"""AttentionPool Trainium2 kernel.

Computes, for x (B,T,m), W1 (m,m), W2 (m,m), vm (1,m):
    h      = tanh(x @ W1 + vm @ W2)          (B,T,m)
    scores = h @ vm[0]                       (B,T,1)
    w      = softmax(scores, axis=T)
    out    = sum(x * w, axis=T, keepdims)    (B,1,m)

Sharding: data-parallel over B across 8 NeuronCores (2 examples per core);
W1/W2/vm replicated.  Softmax needs no max pass: |scores| <= ||vm||_1 (~13
at this scale); with a fixed shift K, e' = exp(s-K) stays inside fp16 range
(overflow would need s > 13.09 > ||vm||_1), so a single streaming pass with
online accumulation of e and Z suffices; the shift cancels in acc/Z.

Per-core dataflow (chunk = 1024 rows of T, t = c*1024 + p*8 + r so each DMA
descriptor is 8 KiB contiguous):
  SWDGE cast-DMA x chunk f32->fp16 (one load, used by both paths)
  -> PE transpose (fp16) per mh -> xts in SBUF (DVE psum->sbuf copy)
  -> h^T = W1.T @ x^T per n-half (PE fp16, 512-col tiles)
  -> tanh + per-partition bias c = vm@W2 (ACT, one instr per n-half)
  -> scores: s = h^T.T @ vm per 128-t block (PE, h stationary)
  -> e16 = exp(s - K) (ACT, fp16) with fused accum_out -> Z partials
  -> pooling on PE: acc[1,m] += e16[t] * x[t,m] via lhsT=e16 column,
     rhs=x chunk rows, accumulated in PSUM across the whole example
  -> tail: Z = partition-reduce of Z partials (PE), 1/Z (DVE), scale, DMA.
"""

import numpy as np

import concourse.bass as bass
import concourse.tile as tile
from concourse import bacc, mybir
from concourse.bass_utils import run_bass_kernel_spmd
from concourse.masks import make_identity

FP32 = mybir.dt.float32
FP16 = mybir.dt.float16
AF = mybir.ActivationFunctionType

N_CORES = 8
B = 16
B_PER_CORE = B // N_CORES  # 2
T = 8192
M = 256
P = 128
CHUNK = 1024         # t rows per chunk
NT = CHUNK // P      # 8 t-tiles (r values) per chunk
NCHUNK = T // CHUNK  # 8 chunks per example
KSUB = 2.0           # exp shift: e' = exp(s - KSUB), cancels in acc/Z


def _build_program() -> bass.Bass:
    nc = bacc.Bacc("TRN2", target_bir_lowering=False, debug=False)

    x = nc.dram_tensor("x", [B_PER_CORE, T, M], FP32, kind="ExternalInput")
    W1 = nc.dram_tensor("W1", [M, M], FP32, kind="ExternalInput")
    W2 = nc.dram_tensor("W2", [M, M], FP32, kind="ExternalInput")
    vm = nc.dram_tensor("vm", [1, M], FP32, kind="ExternalInput")
    out = nc.dram_tensor("out", [B_PER_CORE, M], FP32, kind="ExternalOutput")

    with tile.TileContext(nc) as tc:
        with (
            tc.tile_pool(name="setup", bufs=1) as setup,
            tc.tile_pool(name="xin", bufs=5) as xin_pool,
            tc.tile_pool(name="xtp", bufs=1, space="PSUM") as xtp_pool,
            tc.tile_pool(name="xts", bufs=2) as xts_pool,
            tc.tile_pool(name="hps", bufs=1, space="PSUM") as hps_pool,
            tc.tile_pool(name="hsb", bufs=2) as hsb_pool,
            tc.tile_pool(name="sps", bufs=1, space="PSUM") as sps_pool,
            tc.tile_pool(name="acc", bufs=1, space="PSUM") as acc_pool,
            tc.tile_pool(name="eee", bufs=2) as e_pool,
            tc.tile_pool(name="fin", bufs=2) as fin_pool,
        ):
            # ---------------- setup ----------------
            ident = setup.tile([P, P], FP16)
            make_identity(nc, ident)

            # W1 blocks: w1b[p, mh, n] = W1[mh*128+p, n], cast to fp16
            w1f = setup.tile([P, 2, M], FP32)
            nc.sync.dma_start(out=w1f, in_=W1.rearrange("(a p) n -> p a n", p=P))
            w1b = setup.tile([P, 2, M], FP16)
            nc.vector.tensor_copy(w1b, w1f)

            # W2 blocks (f32, setup only)
            w2f = setup.tile([P, 2, M], FP32)
            nc.sync.dma_start(out=w2f, in_=W2.rearrange("(a p) n -> p a n", p=P))

            # vm transposed: vmt[p, mh] = vm[0, mh*128+p]
            vmt_f = setup.tile([P, 2], FP32)
            nc.sync.dma_start(out=vmt_f, in_=vm[0].rearrange("(a p) -> p a", p=P))
            vmt_b = setup.tile([P, 2], FP16)
            nc.vector.tensor_copy(vmt_b, vmt_f)

            # c = vm @ W2, computed directly transposed: c_sb[p, nh] = c[nh*128+p]
            c_ps = sps_pool.tile([P, 2], FP32, tag="sps")
            for nh in range(2):
                for mh in range(2):
                    nc.tensor.matmul(
                        c_ps[:, nh : nh + 1],
                        lhsT=w2f[:, mh, nh * P : (nh + 1) * P],
                        rhs=vmt_f[:, mh : mh + 1],
                        start=(mh == 0),
                        stop=(mh == 1),
                    )
            c_sb = setup.tile([P, 2], FP32)
            nc.vector.tensor_copy(c_sb, c_ps)

            ones_col = setup.tile([P, 1], FP32)
            nc.vector.memset(ones_col, 1.0)
            kbias = setup.tile([P, 1], FP32)
            nc.vector.memset(kbias, -KSUB)

            # ---------------- main loop ----------------
            for b in range(B_PER_CORE):
                e16 = e_pool.tile([P, NCHUNK * NT], FP16, tag="e16")
                zcol = e_pool.tile([P, NCHUNK], FP32, tag="zcol")
                acc_ps = acc_pool.tile([1, M], FP32, tag="acc")

                for c in range(NCHUNK):
                    # x chunk: xin[p, r, m] = x[b, c*1024 + p*8 + r, m], cast
                    # f32 -> fp16 during the DMA (SWDGE); 8 KiB contiguous
                    # HBM reads per partition.
                    xin = xin_pool.tile([P, NT, M], FP16)
                    nc.gpsimd.dma_start(
                        out=xin,
                        in_=x[b, c * CHUNK : (c + 1) * CHUNK, :].rearrange(
                            "(p r) m -> p r m", p=P
                        ),
                    )

                    # PE transpose per m-half -> xts[q, r, p] = x[t=p*8+r, mh*128+q]
                    xts = []
                    for mh in range(2):
                        xtp = xtp_pool.tile([P, NT, P], FP16, tag=f"xtp{mh}")
                        for r in range(NT):
                            nc.tensor.transpose(
                                xtp[:, r, :],
                                xin[:, r, mh * P : (mh + 1) * P],
                                ident,
                            )
                        xts_mh = xts_pool.tile([P, NT, P], FP16, tag=f"xts{mh}")
                        nc.vector.tensor_copy(xts_mh, xtp)
                        xts.append(xts_mh)

                    # h^T = W1.T @ x^T per n-half; 512-col tiles (PSUM bank)
                    hsb = []
                    for nh in range(2):
                        hps = hps_pool.tile([P, CHUNK], FP32, tag=f"hps{nh}")
                        for th in range(CHUNK // 512):
                            for mh in range(2):
                                nc.tensor.matmul(
                                    hps[:, th * 512 : (th + 1) * 512],
                                    lhsT=w1b[:, mh, nh * P : (nh + 1) * P],
                                    rhs=xts[mh][:, th * 4 : (th + 1) * 4, :],
                                    start=(mh == 0),
                                    stop=(mh == 1),
                                )
                        # tanh with per-partition bias c, one instr per n-half
                        hsb_nh = hsb_pool.tile([P, CHUNK], FP16, tag=f"hsb{nh}")
                        nc.scalar.activation(
                            hsb_nh,
                            hps,
                            AF.Tanh,
                            bias=c_sb[:, nh : nh + 1],
                        )
                        hsb.append(hsb_nh)

                    # scores: s[q, r] for t = q*8 + r (t-partitioned)
                    sps = sps_pool.tile([P, NT], FP32, tag="sps")
                    for r in range(NT):
                        for nh in range(2):
                            nc.tensor.matmul(
                                sps[:, r : r + 1],
                                lhsT=hsb[nh][:, r * P : (r + 1) * P],
                                rhs=vmt_b[:, nh : nh + 1],
                                start=(nh == 0),
                                stop=(nh == 1),
                            )

                    # e' = exp(s - K) in fp16, with fused Z partials
                    nc.scalar.activation(
                        e16[:, c * NT : (c + 1) * NT],
                        sps,
                        AF.Exp,
                        bias=kbias,
                        accum_out=zcol[:, c : c + 1],
                    )

                    # pooling: acc[0, m] += sum_p e'[p, c*NT+r] * x[p, r, m]
                    for r in range(NT):
                        nc.tensor.matmul(
                            acc_ps,
                            lhsT=e16[:, c * NT + r : c * NT + r + 1],
                            rhs=xin[:, r],
                            start=(c == 0 and r == 0),
                            stop=(c == NCHUNK - 1 and r == NT - 1),
                            skip_group_check=True,
                        )

                # ---- finalize example ----
                # Z = sum over partitions of per-chunk partials
                z_red = fin_pool.tile([P, 1], FP32)
                nc.vector.reduce_sum(z_red, zcol, axis=mybir.AxisListType.X)
                z_ps = sps_pool.tile([1, 1], FP32, tag="sps")
                nc.tensor.matmul(z_ps, lhsT=z_red, rhs=ones_col, start=True, stop=True)
                rz = fin_pool.tile([1, 1], FP32)
                nc.vector.reciprocal(rz, z_ps)
                outsb = fin_pool.tile([1, M], FP32)
                nc.vector.tensor_scalar_mul(outsb, acc_ps, rz)
                nc.sync.dma_start(out=out[b : b + 1, :], in_=outsb)

    return nc


_PROGRAM_CACHE: list = []


def _get_program() -> bass.Bass:
    if not _PROGRAM_CACHE:
        nc = _build_program()
        nc.finalize()
        _PROGRAM_CACHE.append(nc)
    return _PROGRAM_CACHE[0]


def kernel(x, W1, W2, vm):
    x = np.ascontiguousarray(x, dtype=np.float32)
    W1 = np.ascontiguousarray(W1, dtype=np.float32)
    W2 = np.ascontiguousarray(W2, dtype=np.float32)
    vm = np.ascontiguousarray(vm, dtype=np.float32)

    nc = _get_program()
    core_ids = list(range(N_CORES))
    in_maps = [
        {
            "x": x[i * B_PER_CORE : (i + 1) * B_PER_CORE],
            "W1": W1,
            "W2": W2,
            "vm": vm,
        }
        for i in range(N_CORES)
    ]
    res = run_bass_kernel_spmd(nc, in_maps, core_ids)
    out = np.concatenate([res.results[i]["out"] for i in range(N_CORES)], axis=0)
    return out.reshape(B, 1, M)
